# revision 8
# baseline (speedup 1.0000x reference)
"""TRN2 Bass kernel for nn_DistributionalQNetwork (C51 categorical projection).

kernel(**inputs) takes FULL unsharded numpy inputs (B=131072), returns the
FULL [B, 251] projected distribution matching reference.py.

Design (8-way batch data parallel, 16384 rows/core):
  - MLP on the PE in float32r (TF32-like rounding, bf16-rate at N=512),
    activations kept transposed [feature, row]; bias+leaky-relu fused into ACT
    passes straight out of PSUM; logits transposed back per 128-row chunk via
    PE; exp on ACT with fused row-sum accumulation; softmax normalization
    deferred to the output scale.
  - C51 projection per 128-row tile: b = clip(r + g*z)/dz has sorted keys with
    steps in {0,1}, so per-bin sums are differences of a prefix scan at
    run-end positions; run-end scan values are scattered to their bins by one
    GPSIMD local_scatter per tile (fp32 split into int16 halves interleaved
    even/odd; l- and u-side share the call in disjoint column regions).
  - The reference divides by fp32(0.8); DVE has no divide, so b = s*1.25
    (<=1 ulp high) plus an exact Sterbenz-split boundary test
    (DELTA = s - m*dzh - m*dzl) recovers the reference's exact-integer (l==u)
    mass-doubling semantics.
  - Five custom fused DVE ops (registered into concourse's dve_ops table at
    import) collapse the hot elementwise pipeline.
"""

import sys

sys.path.insert(0, "/opt/trn_rl_repo")
sys.path.insert(0, "/opt/pypackages")

from contextlib import ExitStack

import numpy as np

import concourse.bass as bass
import concourse.tile as tile
from concourse import bacc, mybir
from concourse.masks import make_identity

F32 = mybir.dt.float32
F32R = mybir.dt.float32r
I16 = mybir.dt.int16
I32 = mybir.dt.int32
AL = mybir.AluOpType
AF = mybir.ActivationFunctionType

NCORES = 8
B_FULL = 131072
ROWS = B_FULL // NCORES  # 16384 rows per core
OBS, ACTD, IN, H, NA = 96, 32, 128, 512, 251

# merged scatter destination: [128, NB2] fp32 viewed as [128, 2*NB2] int16.
# l-side bin k lives at fp32 col 1+k (cols 0..256), u-side at col UOFF+k.
UOFF = 263
NB2 = 524
NE2 = 2 * NB2

# exact split of dz = fl(0.8) = 13421773 * 2^-24 for the boundary test
DZI = 13421773
DZH = float((DZI >> 12) * 2.0**-12)
DZL = float((DZI & 0xFFF) * 2.0**-24)
THRC = float(DZI * 2.0**-48)  # dz * 2^-24 (exact fp32)
P23 = float(2**23)

# ---------------------------------------------------------------- custom ops
import concourse.dve_ops as _dve_ops
from concourse.dve_ops import DveOp as _DveOp
from concourse.dve_spec import (
    AluOp as _AluOp,
    Bin as _Bin,
    C0 as _C0,
    C1 as _C1,
    Spec as _Spec,
    Src0 as _Src0,
    Src1 as _Src1,
    Zero as _Zero,
    eq as _eq,
    lower as _lower,
    maxx as _maxx,
    relu as _relu,
    _has_src1,
)
from concourse.dve_uop import DveOpSpec as _DveOpSpec

_F = np.float32


def _register_op(name, spec):
    if name in _dve_ops._SUB_OPCODE_FOR_NAME:
        for op in _dve_ops.OPS:
            if op.name == name:
                return op
    row = max(_dve_ops._SUB_OPCODE_FOR_NAME.values()) + 1
    assert row < 0x20, "custom-DVE opcode rows exhausted"
    _dve_ops._SUB_OPCODE_FOR_NAME[name] = row
    shas = {}
    for ver in ("v3", "v4"):
        try:
            tmp = _DveOpSpec(name=name, opcode=row, uops=_lower(spec, ver=ver),
                             rd1_en=_has_src1(spec))
            shas[ver] = tmp.sha(ver)
        except Exception:
            pass
    op = _DveOp(name, spec, subdim=False, uops_sha=shas)
    _dve_ops.OPS.append(op)
    _dve_ops.CUSTOM_DVE_SPECS[name] = spec
    return op


def _ref_delta(in0, in1, c0, c1, c2):
    a = (in0 * _F(c0)).astype(_F)
    b = (in1 - a).astype(_F)
    c = (in0 * _F(c1)).astype(_F)
    return (b - c).astype(_F)


def _ref_eqm(in0, in1, c0, c1, c2):
    absd = np.abs(in0.astype(_F))
    mask = np.ascontiguousarray(np.asarray(c0, _F)).view(np.int32)
    if mask.size > 1:
        mask = mask.reshape(-1, 1)
    pw = (np.ascontiguousarray(in1.astype(_F)).view(np.int32) & mask).view(_F)
    thr = (pw * _F(c1)).astype(_F)
    return np.maximum((thr > absd).astype(_F), (absd == 0).astype(_F))


def _ref_floorfix(in0, in1, c0, c1, c2):
    return (in0.astype(_F) - (in0 > in1).astype(_F)).astype(_F)


def _ref_axpbm(in0, in1, c0, c1, c2):
    return (((in0 * _F(c0)).astype(_F) + _F(c1)).astype(_F) * in1).astype(_F)


def _ref_diffrelu(in0, in1, c0, c1, c2):
    return np.maximum((in0 - in1).astype(_F), _F(0.0))


OP_DELTA = _register_op(
    "ANT_DELTA", _Spec(body=(_Src1 - _Src0 * _C0) - _Src0 * _C1,
                       reference=_ref_delta))
_absd = _maxx(_Src0, _Zero - _Src0)
OP_EQM = _register_op(
    "ANT_EQM",
    _Spec(body=_maxx(
        _Bin(_AluOp.IS_GT,
             _Bin(_AluOp.MULTIPLY, _Bin(_AluOp.BITWISE_AND, _Src1, _C0), _C1),
             _absd),
        _eq(_absd, _Zero)), reference=_ref_eqm))
OP_FLOORFIX = _register_op(
    "ANT_FLOORFIX", _Spec(body=_Src0 - _Bin(_AluOp.IS_GT, _Src0, _Src1),
                          reference=_ref_floorfix))
OP_AXPBM = _register_op(
    "ANT_AXPBM", _Spec(body=(_Src0 * _C0 + _C1) * _Src1, reference=_ref_axpbm))
OP_DIFFRELU = _register_op(
    "ANT_DIFFRELU", _Spec(body=_relu(_Src0 - _Src1), reference=_ref_diffrelu))


# ---------------------------------------------------------------- the kernel
def _build_kernel(ctx, tc, aps, rows, mode="full", reps=1):
    nc = tc.nc
    n_tiles = rows // 128
    n_blocks = rows // 512

    const = ctx.enter_context(tc.tile_pool(name="const", bufs=1))
    scal = ctx.enter_context(tc.tile_pool(name="scal", bufs=1))
    xpool = ctx.enter_context(tc.tile_pool(name="xpool", bufs=3))
    htile = ctx.enter_context(tc.tile_pool(name="htile", bufs=2))
    ltile = ctx.enter_context(tc.tile_pool(name="ltile", bufs=4))
    expp = ctx.enter_context(tc.tile_pool(name="expp", bufs=8))
    work = ctx.enter_context(tc.tile_pool(name="work", bufs=2))
    outp = ctx.enter_context(tc.tile_pool(name="outp", bufs=4))
    ps_mm = ctx.enter_context(tc.tile_pool(name="ps_mm", bufs=3, space="PSUM"))
    ps_l = ctx.enter_context(tc.tile_pool(name="ps_l", bufs=2, space="PSUM"))
    ps_tr = ctx.enter_context(tc.tile_pool(name="ps_tr", bufs=3, space="PSUM"))

    # ---------------- constants ----------------
    ident = const.tile([128, 128], F32)
    make_identity(nc, ident[:])

    zrep = const.tile([128, NA], F32)
    nc.sync.dma_start(zrep[:], aps["q_support"][None, :].to_broadcast([128, NA]))

    emask = const.tile([128, 1], I32)
    nc.vector.memset(emask[:], 0x7F800000)
    zeros = const.tile([128, NA], F32)
    nc.vector.memset(zeros[:], 0.0)

    def load_f32r(name, shape):
        t0 = const.tile(shape, F32, tag=f"{name}_raw")
        nc.sync.dma_start(t0[:], aps[name][:])
        t1 = const.tile(shape, F32R, tag=f"{name}_r")
        nc.vector.tensor_copy(t1[:], t0[:])
        return t1

    w1 = load_f32r("W1", [128, H])
    w2 = [None] * 4
    w3 = [None] * 4
    for k in range(4):
        t0 = const.tile([128, H], F32, tag=f"w2raw{k}")
        nc.sync.dma_start(t0[:], aps["W2"][128 * k : 128 * (k + 1), :])
        w2[k] = const.tile([128, H], F32R, name=f"w2r{k}", tag=f"w2r{k}")
        nc.vector.tensor_copy(w2[k][:], t0[:])
        t3 = const.tile([128, NA], F32, tag=f"w3raw{k}")
        nc.sync.dma_start(t3[:], aps["W3"][128 * k : 128 * (k + 1), :])
        w3[k] = const.tile([128, NA], F32R, name=f"w3r{k}", tag=f"w3r{k}")
        nc.vector.tensor_copy(w3[k][:], t3[:])

    b1t = const.tile([128, 4], F32)
    nc.sync.dma_start(b1t[:], aps["b1"].rearrange("(m p) -> p m", p=128))
    b2t = const.tile([128, 4], F32)
    nc.sync.dma_start(b2t[:], aps["b2"].rearrange("(m p) -> p m", p=128))
    b3a = const.tile([128, 1], F32)
    nc.sync.dma_start(b3a[:], aps["b3"][0:128][:, None])
    b3b = const.tile([123, 1], F32)
    nc.sync.dma_start(b3b[:], aps["b3"][128:251][:, None])

    rew = scal.tile([128, n_tiles], F32)
    nc.sync.dma_start(rew[:], aps["rewards"].rearrange("(t p) -> p t", p=128))
    boo = scal.tile([128, n_tiles], F32)
    nc.sync.dma_start(boo[:], aps["bootstrap"].rearrange("(t p) -> p t", p=128))
    dis = scal.tile([128, n_tiles], F32)
    nc.sync.dma_start(dis[:], aps["discount"].rearrange("(t p) -> p t", p=128))
    gsc = scal.tile([128, n_tiles], F32)
    nc.vector.tensor_tensor(out=gsc[:], in0=boo[:], in1=dis[:], op=AL.mult)

    obs_v = aps["obs"]
    act_v = aps["actions"]
    out_v = aps["proj"]
    emaskf = emask[:].bitcast(F32)

    def phase_b(b):
        """MLP for rows [512b, 512b+512) -> (EXPR, SE) per 128-row chunk."""
        r0 = 512 * b
        xt = xpool.tile([128, 512], F32R, tag="xt")
        for c in range(4):
            xc = xpool.tile([128, 128], F32, tag="xc")
            nc.sync.dma_start(xc[:, 0:OBS], obs_v[r0 + 128 * c : r0 + 128 * (c + 1), :])
            nc.sync.dma_start(
                xc[:, OBS:IN], act_v[r0 + 128 * c : r0 + 128 * (c + 1), :]
            )
            tp = ps_tr.tile([128, 128], F32, space="PSUM", tag="pstr", name="tp_x")
            nc.tensor.transpose(tp[:], xc[:], ident[:])
            nc.vector.tensor_copy(xt[:, 128 * c : 128 * (c + 1)], tp[:])

        h1 = [None] * 4
        for m in range(4):
            ps = ps_mm.tile([128, 512], F32, space="PSUM", tag="psmm", name="ps1")
            nc.tensor.matmul(
                ps[:], lhsT=w1[:, 128 * m : 128 * (m + 1)], rhs=xt[:],
                start=True, stop=True,
            )
            h1[m] = htile.tile([128, 512], F32R, name=f"h1_{m}", tag=f"h1_{m}")
            nc.scalar.activation(
                h1[m][:], ps[:], AF.Lrelu, bias=b1t[:, m : m + 1], scale=1.0,
                alpha=0.01,
            )
        h2 = [None] * 4
        for m in range(4):
            ps = ps_mm.tile([128, 512], F32, space="PSUM", tag="psmm", name="ps2")
            for k in range(4):
                nc.tensor.matmul(
                    ps[:], lhsT=w2[k][:, 128 * m : 128 * (m + 1)], rhs=h1[k][:],
                    start=(k == 0), stop=(k == 3),
                )
            h2[m] = htile.tile([128, 512], F32R, name=f"h2_{m}", tag=f"h2_{m}")
            nc.scalar.activation(
                h2[m][:], ps[:], AF.Lrelu, bias=b2t[:, m : m + 1], scale=1.0,
                alpha=0.01,
            )
        lt0 = ltile.tile([128, 512], F32, tag="lt0")
        lt1 = ltile.tile([123, 512], F32, tag="lt1")
        for m, (lt, bb, w) in enumerate(((lt0, b3a, 128), (lt1, b3b, 123))):
            ps = ps_l.tile([128, 512], F32, space="PSUM", tag="psL",
                           name=f"psL{m}")[0:w, :]
            for k in range(4):
                nc.tensor.matmul(
                    ps[:], lhsT=w3[k][:, 128 * m : 128 * m + w], rhs=h2[k][:],
                    start=(k == 0), stop=(k == 3),
                )
            nc.scalar.activation(lt[:], ps[:], AF.Identity, bias=bb[:], scale=1.0)

        res = []
        for c in range(4):
            t0 = ps_tr.tile([128, 128], F32, space="PSUM", tag="pstr", name="tL0")
            nc.tensor.transpose(t0[:], lt0[:, 128 * c : 128 * (c + 1)], ident[:])
            t1 = ps_tr.tile([128, 128], F32, space="PSUM", tag="pstr",
                            name="tL1")[:, 0:123]
            nc.tensor.transpose(
                t1[:], lt1[:, 128 * c : 128 * (c + 1)], ident[0:123, 0:123]
            )
            ex = expp.tile([128, NA], F32, tag="ex")
            se = expp.tile([128, 2], F32, tag="se")
            nc.scalar.activation(
                ex[:, 0:128], t0[:], AF.Exp, bias=0.0, scale=1.0,
                accum_out=se[:, 0:1],
            )
            nc.scalar.activation(
                ex[:, 128:NA], t1[:], AF.Exp, bias=0.0, scale=1.0,
                accum_out=se[:, 1:2],
            )
            res.append((ex, se))
        return res

    def phase_c(t, ex, se):
        """C51 projection for row-tile t (rows [128t, 128t+128))."""
        g_sc = gsc[:, t : t + 1]
        r_sc = rew[:, t : t + 1]

        # b pipeline (rounding-exact wrt reference)
        t1 = work.tile([128, NA], F32, tag="t1")
        nc.scalar.activation(t1[:], zrep[:], AF.Copy, bias=0.0, scale=g_sc)
        t2 = work.tile([128, NA], F32, tag="t2")
        nc.vector.tensor_scalar(out=t2[:], in0=t1[:], scalar1=r_sc, scalar2=100.0,
                                op0=AL.add, op1=AL.add)
        s = work.tile([128, NA], F32, tag="s")
        nc.vector.tensor_scalar(out=s[:], in0=t2[:], scalar1=0.0, scalar2=200.0,
                                op0=AL.max, op1=AL.min)
        b0 = work.tile([128, NA], F32, tag="b0")
        nc.scalar.activation(b0[:], s[:], AF.Copy, bias=0.0, scale=1.25)

        mm = work.tile([128, NA], F32, tag="mm")
        nc.vector.tensor_scalar(out=mm[:], in0=b0[:], scalar1=P23, scalar2=P23,
                                op0=AL.add, op1=AL.subtract)
        delta = work.tile([128, NA], F32, tag="delta")
        nc.vector._custom_dve(OP_DELTA, out=delta[:], in0=mm[:], in1=s[:],
                              s0=DZH, s1=DZL)
        eqm = work.tile([128, NA], F32, tag="eqm")
        nc.vector._custom_dve(OP_EQM, out=eqm[:], in0=delta[:], in1=mm[:],
                              s0=emaskf, s1=THRC)
        lf = work.tile([128, NA], F32, tag="lf")
        nc.vector._custom_dve(OP_FLOORFIX, out=lf[:], in0=mm[:], in1=b0[:])

        frac = work.tile([128, NA], F32, tag="frac")
        nc.vector.tensor_tensor(out=frac[:], in0=b0[:], in1=lf[:], op=AL.subtract)
        e1 = work.tile([128, NA], F32, tag="e1")
        nc.vector.scalar_tensor_tensor(out=e1[:], in0=lf[:], scalar=1.0,
                                       in1=eqm[:], op0=AL.is_ge, op1=AL.mult)
        e2 = work.tile([128, NA], F32, tag="e2")
        nc.vector.scalar_tensor_tensor(out=e2[:], in0=lf[:], scalar=249.0,
                                       in1=eqm[:], op0=AL.is_le, op1=AL.mult)

        s1 = work.tile([128, NA], F32, tag="s1")
        nc.vector.tensor_tensor(out=s1[:], in0=frac[:], in1=eqm[:], op=AL.add)
        s2 = work.tile([128, NA], F32, tag="s2")
        nc.vector.scalar_tensor_tensor(out=s2[:], in0=s1[:], scalar=-1.0,
                                       in1=e2[:], op0=AL.mult, op1=AL.add)
        wl = work.tile([128, NA], F32, tag="wl")
        nc.vector._custom_dve(OP_AXPBM, out=wl[:], in0=s2[:], in1=ex[:],
                              s0=1.0, s1=1.0)
        s4 = work.tile([128, NA], F32, tag="s4")
        nc.vector.tensor_tensor(out=s4[:], in0=frac[:], in1=e1[:], op=AL.add)
        wu = work.tile([128, NA], F32, tag="wu")
        nc.vector.tensor_tensor(out=wu[:], in0=s4[:], in1=ex[:], op=AL.mult)

        l_fin = work.tile([128, NA], F32, tag="l_fin")
        nc.vector.tensor_tensor(out=l_fin[:], in0=lf[:], in1=e1[:], op=AL.subtract)
        v1 = work.tile([128, NA], F32, tag="v1")
        nc.vector.scalar_tensor_tensor(out=v1[:], in0=eqm[:], scalar=-1.0,
                                       in1=e2[:], op0=AL.mult, op1=AL.add)
        u_fin = work.tile([128, NA], F32, tag="u_fin")
        nc.vector.scalar_tensor_tensor(out=u_fin[:], in0=v1[:], scalar=1.0,
                                       in1=lf[:], op0=AL.add, op1=AL.add)

        if mode == "mlp":
            proj = outp.tile([128, NA], F32, tag="proj")
            nc.scalar.activation(proj[:], ex[:], AF.Copy, bias=0.0, scale=1.0)
            nc.sync.dma_start(out_v[128 * t : 128 * (t + 1), :], proj[:])
            return

        # prefix scans into one contiguous buffer (l half, u half)
        cboth = work.tile([128, 2 * NA], F32, tag="cboth")
        nc.vector.tensor_tensor_scan(out=cboth[:, 0:NA], data0=wl[:],
                                     data1=zeros[:], initial=0.0,
                                     op0=AL.add, op1=AL.add)
        nc.vector.tensor_tensor_scan(out=cboth[:, NA : 2 * NA], data0=wu[:],
                                     data1=zeros[:], initial=0.0,
                                     op0=AL.add, op1=AL.add)

        sesum = work.tile([128, 1], F32, tag="sesum")
        nc.vector.tensor_tensor(out=sesum[:], in0=se[:, 0:1], in1=se[:, 1:2],
                                op=AL.add)
        recip = work.tile([128, 1], F32, tag="recip")
        nc.vector.reciprocal(recip[:], sesum[:])

        if mode == "noscatter":
            pall = outp.tile([128, NA], F32, tag="pall")
            nc.vector.tensor_tensor(out=pall[:], in0=cboth[:, 0:NA],
                                    in1=cboth[:, NA : 2 * NA], op=AL.add)
            proj = outp.tile([128, NA], F32, tag="proj")
            nc.scalar.activation(proj[:], pall[:], AF.Copy, bias=0.0,
                                 scale=recip[:])
            nc.sync.dma_start(out_v[128 * t : 128 * (t + 1), :], proj[:])
            return

        # run-end masks + interleaved int16 index pairs for both sides
        idxI = work.tile([128, 4 * NA], I16, tag="idxI")
        iv = idxI[:].rearrange("p (n two) -> p n two", two=2)
        for side, keys, base in (("l", l_fin, 4.0), ("u", u_fin, 2.0 * UOFF + 2.0)):
            rend = work.tile([128, NA], F32, tag=f"rend{side}", name=f"rend{side}")
            nc.vector.tensor_tensor(out=rend[:, 0 : NA - 1], in0=keys[:, 0 : NA - 1],
                                    in1=keys[:, 1:NA], op=AL.not_equal)
            nc.vector.memset(rend[:, NA - 1 : NA], 1.0)
            idxf0 = work.tile([128, NA], F32, tag=f"idxf0{side}",
                              name=f"idxf0{side}")
            nc.vector._custom_dve(OP_AXPBM, out=idxf0[:], in0=keys[:],
                                  in1=rend[:], s0=2.0, s1=base)
            off = 0 if side == "l" else NA
            nc.scalar.activation(iv[:, off : off + NA, 0], idxf0[:], AF.Copy,
                                 bias=-2.0, scale=1.0)
            nc.scalar.activation(iv[:, off : off + NA, 1], idxf0[:], AF.Copy,
                                 bias=-1.0, scale=1.0)

        dst = work.tile([128, NE2], I16, tag="dst")
        nc.gpsimd.local_scatter(
            out_ap=dst[:], data_ap=cboth[:].bitcast(I16), idxs_ap=idxI[:],
            channels=128, num_elems=NE2, num_idxs=4 * NA,
        )

        D = dst[:].bitcast(F32)
        pl = work.tile([128, NA], F32, tag="pl")
        nc.vector._custom_dve(OP_DIFFRELU, out=pl[:], in0=D[:, 1 : NA + 1],
                              in1=D[:, 0:NA])
        pu = work.tile([128, NA], F32, tag="pu")
        nc.vector._custom_dve(OP_DIFFRELU, out=pu[:], in0=D[:, UOFF : UOFF + NA],
                              in1=D[:, UOFF - 1 : UOFF - 1 + NA])
        pall = outp.tile([128, NA], F32, tag="pall")
        nc.vector.tensor_tensor(out=pall[:], in0=pl[:], in1=pu[:], op=AL.add)
        proj = outp.tile([128, NA], F32, tag="proj")
        nc.scalar.activation(proj[:], pall[:], AF.Copy, bias=0.0, scale=recip[:])
        nc.sync.dma_start(out_v[128 * t : 128 * (t + 1), :], proj[:])

    for _ in range(reps):
        for b in range(n_blocks):
            res = phase_b(b)
            for c in range(4):
                ex, se = res[c]
                phase_c(4 * b + c, ex, se)


def build_program(rows=ROWS, num_devices=NCORES, mode="full", reps=1):
    nc = bacc.Bacc(
        "TRN2",
        target_bir_lowering=False,
        debug=False,
        enable_asserts=True,
        num_devices=num_devices,
    )
    aps = {}
    specs = {
        "obs": [rows, OBS],
        "actions": [rows, ACTD],
        "rewards": [rows],
        "bootstrap": [rows],
        "discount": [rows],
        "q_support": [NA],
        "W1": [IN, H],
        "b1": [H],
        "W2": [H, H],
        "b2": [H],
        "W3": [H, NA],
        "b3": [NA],
    }
    for name, shape in specs.items():
        aps[name] = nc.dram_tensor(name, shape, F32, kind="ExternalInput").ap()
    aps["proj"] = nc.dram_tensor("proj", [rows, NA], F32, kind="ExternalOutput").ap()

    with tile.TileContext(nc) as tc, ExitStack() as ctx:
        _build_kernel(ctx, tc, aps, rows, mode=mode, reps=reps)
    nc.compile()
    return nc


_NC_CACHE = {}


def kernel(**inputs):
    obs = np.ascontiguousarray(np.asarray(inputs["obs"], dtype=np.float32))
    B = obs.shape[0]
    rows = B // NCORES
    if rows not in _NC_CACHE:
        _NC_CACHE[rows] = build_program(rows=rows)
    nc = _NC_CACHE[rows]

    full = {
        k: np.ascontiguousarray(np.asarray(inputs[k], dtype=np.float32))
        for k in (
            "obs", "actions", "rewards", "bootstrap", "discount",
            "q_support", "W1", "b1", "W2", "b2", "W3", "b3",
        )
    }
    shared = ("q_support", "W1", "b1", "W2", "b2", "W3", "b3")
    in_maps = []
    for i in range(NCORES):
        m = {}
        for k in ("obs", "actions", "rewards", "bootstrap", "discount"):
            m[k] = full[k][i * rows : (i + 1) * rows]
        for k in shared:
            m[k] = full[k]
        in_maps.append(m)

    from concourse.bass_utils import run_bass_kernel_spmd

    res = run_bass_kernel_spmd(nc, in_maps, core_ids=list(range(NCORES)))
    out = np.concatenate([res.results[i]["proj"] for i in range(NCORES)], axis=0)
    return out.astype(np.float32)
